# revision 1
# baseline (speedup 1.0000x reference)
"""Trainium2 Bass kernel for nn_Attention_884763263569.

Per-sample compute: k/v projections per view t, q over the concat, 3-way
softmax attention, small FC head.  Pure data-parallel over 8 NeuronCores.

Layout strategy (per core, NB = B/8 samples):
 - host pre-transposes x to bf16 [ngroups, 128, 12, GROUP] (group-major,
   partition-contiguous) so the PE consumes x chunks directly as stationary
   operands (contraction dim on partitions) with zero on-chip transposes of
   the big tensor, and each DMA descriptor is a contiguous 24KB run.
 - weights fused host-side into wc[12, 128, 96]: for chunk c (t = c//4),
   columns are [Wk.T | Wv.T | Wq_t.T] for that 128-row d-range.
 - per 128-sample subtile: 12 matmuls accumulate y = [128 samples, 288]
   (three 96-wide groups [k_t | v_t | qp_t]) in one PSUM bank.
 - epilogue in sample-major layout: q = sum_t qp_t, logits via mul+reduce,
   exp (no max-subtraction: logits bounded ~±30), u = sum_t e_t*v_t with Z
   appended as column 33, PE-transpose of [128,33], FC matmul with bias
   folded through the Z column, 1/Z scale on ScalarE, natural row-major
   output DMA.
"""

import os
import sys
from contextlib import ExitStack

import numpy as np

sys.path.insert(0, "/opt/trn_rl_repo")

import ml_dtypes

import concourse.bass as bass
import concourse.tile as tile
from concourse import mybir
from concourse.bass_utils import run_bass_kernel_spmd
from concourse.masks import make_identity

# bass_utils imports antenv.axon_hooks unguarded when BASS_TRACE is set; some
# images ship an antenv without that module — stub it so tracing degrades
# gracefully instead of crashing.
try:
    import antenv.axon_hooks  # noqa: F401
except ImportError:
    import types

    import antenv

    _hooks = types.ModuleType("antenv.axon_hooks")
    _hooks._h = None
    _hooks.set_axon_ntff_profile_hook = lambda h: setattr(_hooks, "_h", h)
    _hooks.get_axon_ntff_profile_hook = lambda: _hooks._h
    sys.modules["antenv.axon_hooks"] = _hooks
    antenv.axon_hooks = _hooks

BF16 = ml_dtypes.bfloat16

NCORES = 8
T, D, P, C = 3, 512, 32, 10
DF = T * D            # 1536
KC = DF // 128        # 12 d-chunks
GROUP = 1024          # samples per pipeline group
SUB = GROUP // 128    # 128-sample subtiles per group
SLAB = 512            # samples per DMA slab
GPRIO = 150           # ~one group worth of instruction priority
SPG = GROUP // SLAB   # slabs per group


def _ins_dim(ap_obj, pos, size, stride=0):
    """Return a new AP with a [stride, size] dim inserted at position pos."""
    new_ap = [list(d) for d in ap_obj.ap]
    new_ap.insert(pos, [stride, size])
    return bass.AP(tensor=ap_obj.tensor, offset=ap_obj.offset, ap=new_ap)


def build_nc(nb):
    assert nb % GROUP == 0
    ngroups = nb // GROUP

    nc = bass.Bass(target_bir_lowering=False)
    nslabs = nb // SLAB
    xt = nc.declare_dram_parameter(
        "xt", [nslabs, 128, KC, SLAB], mybir.dt.bfloat16, isOutput=False
    )
    wc = nc.declare_dram_parameter("wc", [128, KC, 96], mybir.dt.bfloat16, isOutput=False)
    wfc = nc.declare_dram_parameter("wfc", [P + 1, C], mybir.dt.float32, isOutput=False)
    out = nc.declare_dram_parameter("out", [nb, C], mybir.dt.float32, isOutput=True)

    f32 = mybir.dt.float32
    bf16 = mybir.dt.bfloat16
    mult = mybir.AluOpType.mult
    add = mybir.AluOpType.add

    with ExitStack() as ctx:
        tc = ctx.enter_context(tile.TileContext(nc))
        wpool = ctx.enter_context(tc.tile_pool(name="wpool", bufs=1))
        xpool = ctx.enter_context(tc.tile_pool(name="xpool", bufs=8))
        ypsum = ctx.enter_context(tc.tile_pool(name="ypsum", bufs=4, space="PSUM"))
        cpsum = ctx.enter_context(tc.tile_pool(name="cpsum", bufs=1, space="PSUM"))
        opsum = ctx.enter_context(tc.tile_pool(name="opsum", bufs=2, space="PSUM"))
        ypool = ctx.enter_context(tc.tile_pool(name="ypool", bufs=3))
        spool = ctx.enter_context(tc.tile_pool(name="spool", bufs=3))
        opool = ctx.enter_context(tc.tile_pool(name="opool", bufs=3))

        # --- persistent tiles ---
        wc_sb = wpool.tile([128, KC, 96], bf16)
        nc.sync.dma_start(out=wc_sb[:], in_=wc.ap())
        wfc_sb = wpool.tile([P + 1, C], f32)
        nc.sync.dma_start(out=wfc_sb[:], in_=wfc.ap())
        ident = wpool.tile([128, 128], f32)
        make_identity(nc, ident[:])

        out_ap = out.ap()
        xt_ap = xt.ap()

        prev_exp = None
        prev_ct = None
        for g in range(ngroups):
            gs = g * GROUP
            # --- load x^T slabs: [128 (d-in-chunk), KC, SLAB] bf16 ---
            slabs = []
            for h in range(SPG):
                xs = xpool.tile([128, KC, SLAB], bf16)
                nc.sync.dma_start(out=xs[:], in_=xt_ap[g * SPG + h])
                slabs.append(xs)

            # --- projections: per 128-sample subtile, 12 matmuls -> y [128, 288]
            y_list = []
            for j in range(SUB):
                y_ps = ypsum.tile([128, 3 * 96], f32)
                xs = slabs[(j * 128) // SLAB]
                jj = (j * 128) % SLAB
                for c in range(KC):
                    t = c // 4
                    nc.tensor.matmul(
                        y_ps[:, 96 * t : 96 * t + 96],
                        xs[:, c, jj : jj + 128],
                        wc_sb[:, c, :],
                        start=(c % 4 == 0),
                        stop=(c % 4 == 3),
                    )
                y_list.append(y_ps)

            # --- copy y PSUM -> SBUF (bf16), split DVE/ACT ---
            Y = ypool.tile([128, SUB, 288], bf16)
            copy_insts = []
            with tc.high_priority(offset=GPRIO):
                for j, y_ps in enumerate(y_list):
                    copy_insts.append(nc.scalar.copy(out=Y[:, j, :], in_=y_ps[:, :]))
            # order the previous group's ACT tail behind this group's
            # PSUM-freeing copies so the ACT queue never head-of-line blocks
            if prev_exp is not None:
                bass._add_dep_helper(prev_exp.ins, copy_insts[3].ins, reason="exp after next-group copies 0-3")
            if prev_ct is not None:
                bass._add_dep_helper(prev_ct.ins, copy_insts[7].ins, reason="ct after next-group copies 4-7")

            # layout per subtile: [k0 v0 qp0 | k1 v1 qp1 | k2 v2 qp2] blocks of 96
            # (block t at 96*t: k at +0, v at +32, qp at +64)
            # --- q = qp0 + qp1 + qp2 (into the qp0 slot, cols 64:96) ---
            nc.vector.tensor_tensor(
                out=Y[:, :, 64:96], in0=Y[:, :, 64:96], in1=Y[:, :, 160:192], op=add
            )
            nc.vector.tensor_tensor(
                out=Y[:, :, 64:96], in0=Y[:, :, 64:96], in1=Y[:, :, 256:288], op=add
            )

            # --- logits_t = sum_p q*k_t ---
            m_scr = spool.tile([128, SUB, 3, 32], f32)
            q_b = _ins_dim(Y[:, :, 64:96], 2, 3, 0)        # [128, SUB, 3, 32], t bcast
            k_v = _ins_dim(Y[:, :, 0:32], 2, 3, 96)        # k_t at 96*t
            nc.vector.tensor_tensor(out=m_scr[:], in0=q_b, in1=k_v, op=mult)
            logits = spool.tile([128, SUB, 3], f32)
            nc.vector.tensor_reduce(
                out=logits[:], in_=m_scr[:], axis=mybir.AxisListType.X, op=add
            )

            # --- e = exp(logits), Z = sum_t e, R = 1/Z ---
            E = spool.tile([128, SUB, 3], f32)
            prev_exp = nc.scalar.activation(
                out=E[:], in_=logits[:], func=mybir.ActivationFunctionType.Exp
            )
            Z = spool.tile([128, SUB, 1], f32)
            nc.vector.tensor_reduce(
                out=Z[:], in_=E[:], axis=mybir.AxisListType.X, op=add
            )
            R = spool.tile([128, SUB, 1], f32)
            nc.vector.reciprocal(out=R[:], in_=Z[:])

            # --- u = sum_t e_t * v_t ; U33 = [u | Z] ---
            s_scr = spool.tile([128, SUB, 32, 3], f32)
            v_v = _ins_dim(Y[:, :, 32:64], 3, 3, 96)       # dims (g, o, t)
            e_b = _ins_dim(E[:, :, :], 2, 32, 0)           # dims (g, o, t)
            nc.vector.tensor_tensor(out=s_scr[:], in0=v_v, in1=e_b, op=mult)
            U33 = spool.tile([128, SUB, P + 1], f32)
            nc.vector.tensor_reduce(
                out=U33[:, :, 0:32], in_=s_scr[:], axis=mybir.AxisListType.X, op=add
            )
            nc.vector.tensor_copy(out=U33[:, :, 32:33], in_=Z[:])

            # --- cT = U33^T via PE transpose; FC; scale by R ---
            # deferred by ~one group of priority (staggered per transpose so
            # they interleave with the next group's projection stream instead
            # of clumping) so the PE queue never stalls on the DVE chain
            ctx_prio = tc.high_priority(offset=-GPRIO)
            ctx_prio.__enter__()
            ct_ps = cpsum.tile([P + 1, GROUP], f32)
            for j in range(SUB):
                nc.tensor.transpose(
                    ct_ps[:, j * 128 : (j + 1) * 128], U33[:, j, :], ident[:]
                )
            ct_sb = spool.tile([P + 1, GROUP], f32)
            prev_ct = nc.scalar.copy(out=ct_sb[:], in_=ct_ps[:])

            o_ps = opsum.tile([128, SUB, C], f32)
            for j in range(SUB):
                nc.tensor.matmul(
                    o_ps[:, j, :],
                    ct_sb[:, j * 128 : (j + 1) * 128],
                    wfc_sb[:],
                    start=True,
                    stop=True,
                )
            out_sb = opool.tile([128, SUB, C], f32)
            r_b = _ins_dim(R[:, :, 0], 2, C, 0)            # [128, SUB, C], bcast over C
            nc.vector.tensor_tensor(out=out_sb[:], in0=o_ps[:], in1=r_b, op=mult)

            nc.scalar.dma_start(
                out=out_ap[gs : gs + GROUP, :].rearrange("(j p) c -> p j c", p=128),
                in_=out_sb[:],
            )
            ctx_prio.__exit__(None, None, None)

    nc.finalize()
    _split_excess_waits(nc)
    return nc


def _split_excess_waits(nc):
    """walrus rejects >1 sync wait on compute instruction structs; hoist the
    extras onto same-engine NoOps inserted just before the offender."""
    exempt = (mybir.InstEventSemaphore,)
    for func in nc.m.functions:
        for blk in func.blocks:
            insts = list(blk.instructions)
            out_list = []
            changed = False
            for inst in insts:
                si = getattr(inst, "sync_info", None)
                ow = list(si.on_wait) if (si is not None and si.on_wait) else []
                if len(ow) > 1 and not isinstance(inst, exempt):
                    for w in ow[:-1]:
                        nop = mybir.InstNoOp(
                            name=nc.get_next_instruction_name(),
                            engine=inst.engine,
                            sync_info=mybir.SyncInfo(on_wait=[w], on_update=[]),
                            bass_nofuse=True,
                        )
                        out_list.append(nop)
                    si.on_wait = [ow[-1]]
                    changed = True
                out_list.append(inst)
            if changed:
                blk.instructions = out_list


_NC_CACHE = {}


def _get_nc(nb):
    if nb not in _NC_CACHE:
        _NC_CACHE[nb] = build_nc(nb)
    return _NC_CACHE[nb]


def _prep_weights(Wk, Wv, Wq, Wfc, bfc):
    WkT = Wk.T.astype(np.float32)   # [512, 32]
    WvT = Wv.T.astype(np.float32)   # [512, 32]
    WqT = Wq.T.astype(np.float32)   # [1536, 32]
    wc = np.zeros((KC, 128, 96), np.float32)
    for c in range(KC):
        t, dsub = divmod(c, 4)
        d512 = slice(dsub * 128, (dsub + 1) * 128)
        rows = slice(c * 128, (c + 1) * 128)
        wc[c, :, 0:32] = WkT[d512]
        wc[c, :, 32:64] = WvT[d512]
        wc[c, :, 64:96] = WqT[rows]
    wc = np.ascontiguousarray(wc.transpose(1, 0, 2)).astype(BF16)
    wfc_aug = np.concatenate(
        [Wfc.T.astype(np.float32), bfc.reshape(1, C).astype(np.float32)], axis=0
    )  # [33, 10]
    return wc, wfc_aug


LAST_RESULT = None


def kernel(x, Wk, Wv, Wq, Wfc, bfc):
    global LAST_RESULT
    x = np.asarray(x, dtype=np.float32)
    Wk = np.asarray(Wk, dtype=np.float32)
    Wv = np.asarray(Wv, dtype=np.float32)
    Wq = np.asarray(Wq, dtype=np.float32)
    Wfc = np.asarray(Wfc, dtype=np.float32)
    bfc = np.asarray(bfc, dtype=np.float32)

    B = x.shape[0]
    assert B % NCORES == 0
    nb = B // NCORES
    nc = _get_nc(nb)
    wc, wfc_aug = _prep_weights(Wk, Wv, Wq, Wfc, bfc)

    xr = x.reshape(NCORES, nb, DF)
    in_maps = []
    for i in range(NCORES):
        # xt[h, p, c, s] = xT[c*128+p, h*SLAB+s]
        xt = np.ascontiguousarray(
            xr[i]
            .astype(BF16)
            .T.reshape(KC, 128, nb // SLAB, SLAB)
            .transpose(2, 1, 0, 3)
        )
        in_maps.append({"xt": xt, "wc": wc, "wfc": wfc_aug})

    LAST_RESULT = run_bass_kernel_spmd(nc, in_maps, core_ids=list(range(NCORES)))
    res = LAST_RESULT.results
    out = np.concatenate([res[i]["out"] for i in range(NCORES)], axis=0)
    return out.astype(np.float32)



# revision 4
# speedup vs baseline: 1.0986x; 1.0986x over previous
"""Trainium2 Bass kernel for nn_Attention_884763263569.

Per-sample compute: k/v projections per view t, q over the concat, 3-way
softmax attention, small FC head.  Pure data-parallel over 8 NeuronCores.

v1 strategy (feature-major stage 1, DMA on two rings):
 - weights are the STATIONARY operand, x streams as the moving operand
   (N=512), so the PE ingests each x element exactly once at stream rate
   instead of paying a 128-cycle stationary load per 128 samples.
 - Wfc is folded into Wv on host (f = (Wfc@Wv) x), so stage-1 banks carry
   [k | qp | f] = 74 useful rows; stationary padded to 128 cols.
 - per 512-sample slab: 12 matmuls accumulate three per-view PSUM banks
   [128 rows, 512 samples]; banks are copied to SBUF bf16, then 12 PE
   transposes ([74,128] blocks) return to sample-major for the epilogue
   (softmax over 3 logits, weighted sum, scale+bias) on DVE/ACT.
 - input DMAs alternate between the SP HWDGE ring (nc.sync) and the
   SWDGE path (nc.gpsimd) so per-DMA completion latency on one ring is
   hidden by data movement on the other; ACT stays free for copies.
 - outputs accumulate in SBUF and leave in ONE contiguous DMA at the end
   ([128, nslabs*4, 10]); the host inverts the sample permutation for free.
"""

import os
import sys
from contextlib import ExitStack

import numpy as np

sys.path.insert(0, "/opt/trn_rl_repo")

import ml_dtypes

import concourse.bass as bass
import concourse.tile as tile
from concourse import mybir
from concourse.bass_utils import run_bass_kernel_spmd
from concourse.masks import make_identity

# bass_utils imports antenv.axon_hooks unguarded when BASS_TRACE is set; some
# images ship an antenv without that module — stub it so tracing degrades
# gracefully instead of crashing.
try:
    import antenv.axon_hooks  # noqa: F401
except ImportError:
    import types

    import antenv

    _hooks = types.ModuleType("antenv.axon_hooks")
    _hooks._h = None
    _hooks.set_axon_ntff_profile_hook = lambda h: setattr(_hooks, "_h", h)
    _hooks.get_axon_ntff_profile_hook = lambda: _hooks._h
    sys.modules["antenv.axon_hooks"] = _hooks
    antenv.axon_hooks = _hooks


def _register_ctypes_ntff_hook():
    """If no NTFF profile hook is registered, drive profiling via direct
    ctypes calls into libaxon_pjrt.so (slim equivalent of axon.trn's hook;
    same C ABI the boot script uses)."""
    import contextlib
    import ctypes

    from antenv.axon_hooks import (
        get_axon_ntff_profile_hook,
        set_axon_ntff_profile_hook,
    )

    if get_axon_ntff_profile_hook() is not None:
        return
    so_path = os.environ.get("AXON_PJRT_SO", "/opt/axon/libaxon_pjrt.so")
    if not os.path.exists(so_path):
        return
    try:
        lib = ctypes.CDLL(so_path)
    except OSError:
        return
    if not hasattr(lib, "axon_start_nrt_profile"):
        return
    lib.axon_start_nrt_profile.argtypes = [
        ctypes.POINTER(ctypes.c_int64),
        ctypes.c_size_t,
    ]
    lib.axon_start_nrt_profile.restype = ctypes.c_int64
    lib.axon_stop_nrt_profile.argtypes = [ctypes.c_char_p]
    lib.axon_stop_nrt_profile.restype = ctypes.c_int64

    @contextlib.contextmanager
    def _hook(output_dir, device_ids):
        import jax

        jax.devices()
        if device_ids:
            ids = (ctypes.c_int64 * len(device_ids))(*device_ids)
            rc = lib.axon_start_nrt_profile(ids, len(device_ids))
        else:
            rc = lib.axon_start_nrt_profile(None, 0)
        if rc != 0:
            raise RuntimeError(f"axon_start_nrt_profile rc={rc}")
        try:
            yield
        finally:
            n = lib.axon_stop_nrt_profile(str(output_dir).encode())
            print(f"ntff profile: {n} file(s) written to {output_dir}", file=sys.stderr)

    set_axon_ntff_profile_hook(_hook)


try:
    _register_ctypes_ntff_hook()
except Exception:
    pass

BF16 = ml_dtypes.bfloat16

NCORES = 8
T, D, P, C = 3, 512, 32, 10
DF = T * D            # 1536
KC = DF // 128        # 12 d-chunks
SLAB = 512            # samples per slab (one matmul moving width)
NW = 74               # useful stationary cols: 32 k + 32 qp + 10 f


def _ins_dim(ap_obj, pos, size, stride=0):
    """Return a new AP with a [stride, size] dim inserted at position pos."""
    new_ap = [list(d) for d in ap_obj.ap]
    new_ap.insert(pos, [stride, size])
    return bass.AP(tensor=ap_obj.tensor, offset=ap_obj.offset, ap=new_ap)


def build_nc(nb):
    assert nb % SLAB == 0
    nslabs = nb // SLAB

    nc = bass.Bass(target_bir_lowering=False)
    xt = nc.declare_dram_parameter(
        "xt", [nslabs, 128, KC, SLAB], mybir.dt.bfloat16, isOutput=False
    )
    wc = nc.declare_dram_parameter("wc", [128, KC, 128], mybir.dt.bfloat16, isOutput=False)
    bfcr = nc.declare_dram_parameter("bfcr", [128, C], mybir.dt.float32, isOutput=False)
    out = nc.declare_dram_parameter(
        "out", [128, nslabs * 4, C], mybir.dt.float32, isOutput=True
    )

    f32 = mybir.dt.float32
    bf16 = mybir.dt.bfloat16
    mult = mybir.AluOpType.mult
    add = mybir.AluOpType.add

    with ExitStack() as ctx:
        tc = ctx.enter_context(tile.TileContext(nc))
        wpool = ctx.enter_context(tc.tile_pool(name="wpool", bufs=1))
        xpool = ctx.enter_context(tc.tile_pool(name="xpool", bufs=6))
        ypsum = ctx.enter_context(tc.tile_pool(name="ypsum", bufs=2, space="PSUM"))
        cpsum = ctx.enter_context(tc.tile_pool(name="cpsum", bufs=1, space="PSUM"))
        ypool = ctx.enter_context(tc.tile_pool(name="ypool", bufs=3))
        typool = ctx.enter_context(tc.tile_pool(name="typool", bufs=3))
        spool = ctx.enter_context(tc.tile_pool(name="spool", bufs=3))
        opool = ctx.enter_context(tc.tile_pool(name="opool", bufs=1))

        # --- persistent tiles ---
        wc_sb = wpool.tile([128, KC, 128], bf16)
        nc.sync.dma_start(out=wc_sb[:], in_=wc.ap())
        bfc_sb = wpool.tile([128, C], f32)
        nc.gpsimd.dma_start(out=bfc_sb[:], in_=bfcr.ap())
        ident = wpool.tile([128, 128], bf16)
        make_identity(nc, ident[:])
        obuf = wpool.tile([128, nslabs * 4, C], f32)

        xt_ap = xt.ap()

        for h in range(nslabs):
            # --- load x^T slab: [128 (d-in-chunk), KC, SLAB] bf16, two rings ---
            xs = xpool.tile([128, KC, SLAB], bf16)
            if h % 2 == 0:
                nc.sync.dma_start(out=xs[:], in_=xt_ap[h])
            else:
                nc.gpsimd.dma_start(out=xs[:], in_=xt_ap[h])

            # --- projections: per view t, 4 chunk-matmuls accumulate one bank
            # y rows: [k 0:32 | qp 32:64 | f 64:74 | pad]
            y_ps = ypsum.tile([128, T, SLAB], f32)
            for t in range(T):
                for i in range(4):
                    c = 4 * t + i
                    nc.tensor.matmul(
                        y_ps[:, t, :],
                        wc_sb[:, c, :],
                        xs[:, c, :],
                        start=(i == 0),
                        stop=(i == 3),
                    )

            # --- evacuate banks to SBUF bf16 (ACT does 2, DVE does 1) ---
            ysb = ypool.tile([128, T, SLAB], bf16)
            nc.scalar.copy(out=ysb[:, 0, :], in_=y_ps[:, 0, :])
            nc.scalar.copy(out=ysb[:, 1, :], in_=y_ps[:, 1, :])
            nc.vector.tensor_copy(out=ysb[:, 2, :], in_=y_ps[:, 2, :])

            # --- 12 PE transposes -> sample-major [128, (b,t), 74] ---
            # j = b*3 + t; bf16 pass-through: all 12*74=888 bf16 fit one bank
            ct_ps = cpsum.tile([128, 12, NW], bf16)
            for b in range(4):
                for t in range(T):
                    j = b * 3 + t
                    nc.tensor.transpose(
                        ct_ps[:, j, :],
                        ysb[0:NW, t, b * 128 : (b + 1) * 128],
                        ident[0:NW, 0:NW],
                    )
            ty = typool.tile([128, 12, NW], bf16)
            nc.vector.tensor_copy(out=ty[:], in_=ct_ps[:])

            # --- epilogue in sample-major; ty viewed as [128, 4(b), 3(t), 74] ---
            def tyv(t, c0, c1):
                # AP [128, 4(b), c1-c0] for view t, cols [c0:c1)
                return _ins_dim(ty[:, t, c0:c1], 1, 4, 3 * NW)

            q = spool.tile([128, 4, P], f32)
            nc.vector.tensor_tensor(out=q[:], in0=tyv(0, 32, 64), in1=tyv(1, 32, 64), op=add)
            nc.vector.tensor_tensor(out=q[:], in0=q[:], in1=tyv(2, 32, 64), op=add)

            # m[b,t,p] = q[b,p] * k[b,t,p]
            m = spool.tile([128, 4, T, P], f32)
            k_ap = _ins_dim(ty[:, 0:3, 0:32], 1, 4, 3 * NW)  # [128, 4, 3, 32]
            q_b = _ins_dim(q[:], 2, T, 0)                     # [128, 4, 3, 32]
            nc.vector.tensor_tensor(out=m[:], in0=q_b, in1=k_ap, op=mult)
            logits = spool.tile([128, 4, T], f32)
            nc.vector.tensor_reduce(
                out=logits[:], in_=m[:], axis=mybir.AxisListType.X, op=add
            )

            # e = exp(logits) (logits bounded ~±35, no max-subtraction needed)
            e = spool.tile([128, 4, T], f32)
            nc.scalar.activation(
                out=e[:], in_=logits[:], func=mybir.ActivationFunctionType.Exp
            )
            z = spool.tile([128, 4, 1], f32)
            nc.vector.tensor_reduce(out=z[:], in_=e[:], axis=mybir.AxisListType.X, op=add)
            r = spool.tile([128, 4, 1], f32)
            nc.vector.reciprocal(out=r[:], in_=z[:])

            # s[b,f,t] = e[b,t] * fmat[b,t,f]  (out written t-innermost)
            s = spool.tile([128, 4, C, T], f32)
            f_ap = _ins_dim(ty[:, 0:3, 64:74], 1, 4, 3 * NW)  # [128, 4, 3, 10]
            e_b = _ins_dim(e[:], 3, C, 0)                      # [128, 4, 3, 10]
            # out AP iterated (b, t, f) but laid out [b, f, t]
            s_out = bass.AP(
                tensor=s[:].tensor,
                offset=s[:].offset,
                ap=[list(s[:].ap[0]), [C * T, 4], [1, T], [T, C]],
            )
            nc.vector.tensor_tensor(out=s_out, in0=e_b, in1=f_ap, op=mult)
            u = spool.tile([128, 4, C], f32)
            nc.vector.tensor_reduce(out=u[:], in_=s[:], axis=mybir.AxisListType.X, op=add)

            # out = u * r + bfc
            un = spool.tile([128, 4, C], f32)
            r_b = _ins_dim(r[:, :, 0], 2, C, 0)
            nc.vector.tensor_tensor(out=un[:], in0=u[:], in1=r_b, op=mult)
            bfc_b = _ins_dim(bfc_sb[:], 1, 4, 0)
            nc.vector.tensor_tensor(
                out=obuf[:, h * 4 : h * 4 + 4, :], in0=un[:], in1=bfc_b, op=add
            )

        nc.sync.dma_start(out=out.ap(), in_=obuf[:])

    nc.finalize()
    _split_excess_waits(nc)
    return nc


def _split_excess_waits(nc):
    """walrus rejects >1 sync wait on compute instruction structs; hoist the
    extras onto same-engine NoOps inserted just before the offender."""
    exempt = (mybir.InstEventSemaphore,)
    for func in nc.m.functions:
        for blk in func.blocks:
            insts = list(blk.instructions)
            out_list = []
            changed = False
            for inst in insts:
                si = getattr(inst, "sync_info", None)
                ow = list(si.on_wait) if (si is not None and si.on_wait) else []
                if len(ow) > 1 and not isinstance(inst, exempt):
                    for w in ow[:-1]:
                        nop = mybir.InstNoOp(
                            name=nc.get_next_instruction_name(),
                            engine=inst.engine,
                            sync_info=mybir.SyncInfo(on_wait=[w], on_update=[]),
                            bass_nofuse=True,
                        )
                        out_list.append(nop)
                    si.on_wait = [ow[-1]]
                    changed = True
                out_list.append(inst)
            if changed:
                blk.instructions = out_list


_NC_CACHE = {}


def _get_nc(nb):
    if nb not in _NC_CACHE:
        _NC_CACHE[nb] = build_nc(nb)
    return _NC_CACHE[nb]


def _prep_weights(Wk, Wv, Wq, Wfc, bfc):
    Wvf = (Wfc.astype(np.float64) @ Wv.astype(np.float64)).astype(np.float32)  # [10,512]
    WkT = Wk.T.astype(np.float32)    # [512, 32]
    WqT = Wq.T.astype(np.float32)    # [1536, 32]
    WvfT = Wvf.T                     # [512, 10]
    wc = np.zeros((KC, 128, 128), np.float32)
    for c in range(KC):
        t, dsub = divmod(c, 4)
        d512 = slice(dsub * 128, (dsub + 1) * 128)
        rows = slice(c * 128, (c + 1) * 128)
        wc[c, :, 0:32] = WkT[d512]
        wc[c, :, 32:64] = WqT[rows]
        wc[c, :, 64:74] = WvfT[d512]
    wc = np.ascontiguousarray(wc.transpose(1, 0, 2)).astype(BF16)  # [128, KC, 128]
    bfcr = np.ascontiguousarray(
        np.broadcast_to(bfc.reshape(1, C).astype(np.float32), (128, C))
    )
    return wc, bfcr


def _pack_x(xr_core, nb):
    # xt[h, p, c, s] = x_cat[h*SLAB + s, 128c + p]
    return np.ascontiguousarray(
        xr_core.astype(BF16)
        .T.reshape(KC, 128, nb // SLAB, SLAB)
        .transpose(2, 1, 0, 3)
    )


def _unpack_out(arr, nb):
    # arr [128, nslabs*4, C]; sample s = h*SLAB + b*128 + p -> arr[p, h*4+b]
    nslabs = nb // SLAB
    return (
        arr.reshape(128, nslabs, 4, C).transpose(1, 2, 0, 3).reshape(nb, C)
    )


LAST_RESULT = None


def kernel(x, Wk, Wv, Wq, Wfc, bfc):
    global LAST_RESULT
    x = np.asarray(x, dtype=np.float32)
    Wk = np.asarray(Wk, dtype=np.float32)
    Wv = np.asarray(Wv, dtype=np.float32)
    Wq = np.asarray(Wq, dtype=np.float32)
    Wfc = np.asarray(Wfc, dtype=np.float32)
    bfc = np.asarray(bfc, dtype=np.float32)

    B = x.shape[0]
    assert B % NCORES == 0
    nb = B // NCORES
    nc = _get_nc(nb)
    wc, bfcr = _prep_weights(Wk, Wv, Wq, Wfc, bfc)

    xr = x.reshape(NCORES, nb, DF)
    in_maps = []
    for i in range(NCORES):
        in_maps.append({"xt": _pack_x(xr[i], nb), "wc": wc, "bfcr": bfcr})

    LAST_RESULT = run_bass_kernel_spmd(nc, in_maps, core_ids=list(range(NCORES)))
    res = LAST_RESULT.results
    out = np.concatenate(
        [_unpack_out(res[i]["out"], nb) for i in range(NCORES)], axis=0
    )
    return out.astype(np.float32)
